# revision 30
# baseline (speedup 1.0000x reference)
"""Trainium2 Bass kernel for nn_LinearAutoDecoder (moe_routing).

Computes, for each row n:
    rgb[n, :] = (X[n, :63] @ W_pos.T + X[n, 63:] @ W_feat.T)[3*cid[n] : 3*cid[n]+3]

Strategy (data-parallel over 8 NeuronCores, rows sharded):
  - Dense GEMM rgbc = X @ [W_pos | W_feat].T on the tensor engine in bf16
    (fp32 PSUM accumulation). X tiles are transposed on-chip via PE
    transpose; weights are transposed/permuted once at startup.
  - Weight columns are pre-permuted j-major (R rows | G rows | B rows) so
    the per-row gather reduces over a contiguous [3, 64] view.
  - Gather: per-row one-hot mask (tensor_scalar is_equal on gpsimd),
    bf16 multiply at DVE 2x, then a batched reduce -> [128, 4, 3].
  - Work is batched over 4-tile "quads" to amortize per-op engine access
    latency, and spread across DVE/ACT/GPSIMD to stay under the DMA floor.
"""

import os
from contextlib import ExitStack

import numpy as np

import concourse.bass as bass
import concourse.tile as tile
from concourse import bacc, mybir
from concourse.masks import make_identity

P = 128          # SBUF partitions
POS = 63
LAT = 256
K = POS + LAT    # 319 contraction dim
KP = 384         # k padded to 3*128
C = 192          # 3 * 64 clusters
N_CORES = 8
KS = [0, 128, 256]      # k-chunk starts (all chunks 128 wide, last zero-padded)
G = 8            # tiles per DMA batch (contiguous rows per partition)
Q = 4            # tiles per compute quad

f32 = mybir.dt.float32
bf16 = mybir.dt.bfloat16
i32 = mybir.dt.int32
Alu = mybir.AluOpType
Axis = mybir.AxisListType


def build_kernel(T: int, reps: int = 1, stage: str = "e"):
    """Build the single-core Bass program; each core handles rows = 128*T.

    Row mapping on a core: global row r = T*p + t  (partition p, tile t).
    reps > 1 repeats the whole body (for timing-by-differencing).
    """
    rows = P * T
    nc = bacc.Bacc(
        "TRN2",
        target_bir_lowering=False,
        debug=False,
        enable_asserts=False,
    )
    X = nc.dram_tensor("x", [rows, K], f32, kind="ExternalInput").ap()
    CID = nc.dram_tensor("cid", [rows], i32, kind="ExternalInput").ap()
    WP = nc.dram_tensor("w_pos", [C, POS], f32, kind="ExternalInput").ap()
    WF = nc.dram_tensor("w_feat", [C, LAT], f32, kind="ExternalInput").ap()
    OUT = nc.dram_tensor("out", [rows, 3], f32, kind="ExternalOutput").ap()

    with tile.TileContext(nc) as tc, ExitStack() as ctx:
        _body(ctx, tc, X, CID, WP, WF, OUT, T, reps=reps, stage=stage)
    nc.compile()
    return nc


def _body(ctx, tc, X, CID, WP, WF, OUT, T, reps=1, stage="e"):
    """stage: a=DMA only, b=+cast, c=+transpose/copy, d=+matmul+rgbc, e=full."""
    nc = tc.nc

    Xv = X.rearrange("(p t) k -> p t k", p=P)        # [128, T, 319]
    CIDv = CID.rearrange("(p t) -> p t", p=P)        # [128, T]
    OUTv = OUT.rearrange("(p t) j -> p t j", p=P)    # [128, T, 3]

    const = ctx.enter_context(tc.tile_pool(name="const", bufs=1))
    ps_x = ctx.enter_context(tc.tile_pool(name="ps_x", bufs=2, space="PSUM"))
    ps_r = ctx.enter_context(tc.tile_pool(name="ps_r", bufs=2, space="PSUM"))

    # --- one-time setup -------------------------------------------------
    ident = const.tile([P, P], bf16)
    make_identity(nc, ident[:])

    # ramp[p, j*64 + c] = c  (j-major layout)
    ramp_i = const.tile([P, C], i32)
    nc.gpsimd.iota(ramp_i[:], pattern=[[0, 3], [1, 64]], base=0, channel_multiplier=0)
    ramp = const.tile([P, C], bf16)
    nc.vector.tensor_copy(ramp[:], ramp_i[:])

    # cluster ids as fp32 (is_equal scalar operand must be fp32)
    cid_i = const.tile([P, T], i32)
    nc.sync.dma_start(cid_i[:], CIDv)
    cid_f = const.tile([P, T], f32)
    nc.vector.tensor_copy(cid_f[:], cid_i[:])

    # Weights: wt[i] = [128, 192] bf16 = k-chunk i of transpose of
    # bf16([W_pos | W_feat]), zero-padded in k beyond 319, with the 192
    # columns permuted j-major: new_col = (old % 3)*64 + old//3.
    # Split the 192 rows at a multiple-of-3 boundary (126 | 66) so the
    # permutation is an affine access pattern on each piece.
    wt = [
        const.tile([P, C], bf16, tag=f"wt{i}", name=f"wt{i}")
        for i in range(3)
    ]
    nc.vector.memset(wt[2][:], 0.0)
    for r0, pr in [(0, 126), (126, 66)]:
        cbase = r0 // 3
        ngrp = pr // 3
        wpos_s = const.tile([pr, POS], f32, tag=f"wpos{r0}")
        wfeat_s = const.tile([pr, LAT], f32, tag=f"wfeat{r0}")
        nc.sync.dma_start(wpos_s[:], WP[r0 : r0 + pr, :])
        nc.sync.dma_start(wfeat_s[:], WF[r0 : r0 + pr, :])
        wcat = const.tile([pr, K], bf16, tag=f"wcat{r0}")
        nc.vector.tensor_copy(wcat[:, :POS], wpos_s[:])
        nc.vector.tensor_copy(wcat[:, POS:], wfeat_s[:])
        for i, k0 in enumerate(KS):
            kw = min(128, K - k0)
            pw = ps_x.tile([P, Q, KP], bf16, tag="px4", name="pw")
            nc.tensor.transpose(pw[:kw, 0, :pr], wcat[:, k0 : k0 + kw], ident[:pr, :pr])
            # pw[k, q] with q = 3*c + j  ->  wt[i][k, j*64 + cbase + c]
            src = pw[:kw, 0, :pr].rearrange("k (c j) -> k c j", j=3)
            dst = wt[i][:kw].rearrange("k (j c) -> k c j", j=3)[:, cbase : cbase + ngrp, :]
            nc.scalar.copy(dst, src)

    # --- main loop ------------------------------------------------------
    xin = ctx.enter_context(tc.tile_pool(name="xin", bufs=4))
    xtp = ctx.enter_context(tc.tile_pool(name="xt", bufs=4))
    rgbp = ctx.enter_context(tc.tile_pool(name="rgb", bufs=4))
    maskp = ctx.enter_context(tc.tile_pool(name="mask", bufs=4))
    selp = ctx.enter_context(tc.tile_pool(name="sel", bufs=4))
    accp = ctx.enter_context(tc.tile_pool(name="acc", bufs=1))
    rgb_all = accp.tile([P, T, 3], f32)

    # persistent ring for bf16 X: pad region [319:384] zeroed once, feeds
    # the zero-padded third k-chunk of every transpose
    NRING = 4
    xb_ring = accp.tile([P, NRING, G, KP], bf16)
    nc.gpsimd.memset(xb_ring[:, :, :, K:], 0.0)
    if stage == "n":
        nc.gpsimd.memset(rgb_all[:], 0.0)

    assert T % G == 0 and G % Q == 0
    n_iters = 0 if stage == "n" else reps * (T // G)
    for g in range(n_iters):
        g = g % (T // G)
        # one batched load: per partition G consecutive rows -> one
        # contiguous (G*1276)B descriptor instead of G separate ones
        xf = xin.tile([P, G, K], f32, tag="x")
        nc.sync.dma_start(xf[:], Xv[:, g * G : (g + 1) * G, :])
        xb = xb_ring[:, g % NRING]
        if stage >= "b":
            # cast per quad for finer pipelining
            for q in range(G // Q):
                nc.gpsimd.tensor_copy(
                    xb[:, q * Q : (q + 1) * Q, :K], xf[:, q * Q : (q + 1) * Q, :]
                )

        for q in range(G // Q):
            t0 = g * G + q * Q
            if stage == "a":
                nc.vector.tensor_copy(
                    rgb_all[:, t0 : t0 + Q, :], xf[:, q * Q : (q + 1) * Q, :3]
                )
                continue
            if stage == "b":
                nc.vector.tensor_copy(
                    rgb_all[:, t0 : t0 + Q, :], xb[:, q * Q : (q + 1) * Q, :3]
                )
                continue
            # transpose 4 tiles into one 2-bank PSUM tile
            px4 = ps_x.tile([P, Q, KP], bf16, tag="px4")
            for v in range(Q):
                u = q * Q + v
                for i, k0 in enumerate(KS):
                    nc.tensor.transpose(
                        px4[:, v, i * P : (i + 1) * P],
                        xb[:, u, k0 : k0 + P],
                        ident[:],
                    )
            # PSUM -> SBUF; ~1/4 of quads on DVE, rest on ACT (load balance)
            xt4 = xtp.tile([P, Q, KP], bf16, tag="xt")
            if (g * (G // Q) + q) % 4 == 0:
                nc.vector.tensor_copy(xt4[:], px4[:])
            else:
                nc.scalar.copy(xt4[:], px4[:])
            if stage == "c":
                nc.vector.tensor_copy(rgb_all[:, t0 : t0 + Q, :], xt4[:, :, :3])
                continue

            # free dim padded to 256 so each tile's [*, 192] output stays
            # inside one 2KB PSUM bank
            pr4 = ps_r.tile([P, Q, 256], f32, tag="pr4")
            for v in range(Q):
                for i in range(3):
                    nc.tensor.matmul(
                        pr4[:, v, :C],
                        xt4[:, v, i * P : (i + 1) * P],
                        wt[i][:],
                        start=(i == 0),
                        stop=(i == 2),
                    )

            if stage == "p":
                # predicated gather straight from PSUM: for each (q, j) the
                # 64 candidate clusters all target the same output slot via a
                # broadcast write AP; exactly one predicate fires per row.
                maskp4 = maskp.tile([P, Q, C], bf16, tag="mask")
                for v in range(Q):
                    t = t0 + v
                    nc.vector.tensor_scalar(
                        out=maskp4[:, v, :],
                        in0=ramp[:],
                        scalar1=cid_f[:, t : t + 1],
                        scalar2=None,
                        op0=Alu.is_equal,
                    )
                out_b = (
                    rgb_all[:, t0 : t0 + Q, :]
                    .unsqueeze(3)
                    .broadcast_to([P, Q, 3, 64])
                )
                nc.vector.copy_predicated(
                    out_b,
                    maskp4[:].rearrange("p q (j c) -> p q j c", j=3),
                    pr4[:, :, :C].rearrange("p q (j c) -> p q j c", j=3),
                )
                continue

            rgbc4 = rgbp.tile([P, Q, C], bf16, tag="rgbc")
            nc.scalar.copy(rgbc4[:], pr4[:, :, :C])
            if stage == "d":
                nc.vector.tensor_copy(rgb_all[:, t0 : t0 + Q, :], rgbc4[:, :, :3])
                continue

            mask4 = maskp.tile([P, Q, C], bf16, tag="mask")
            for v in range(Q):
                t = t0 + v
                nc.vector.tensor_scalar(
                    out=mask4[:, v, :],
                    in0=ramp[:],
                    scalar1=cid_f[:, t : t + 1],
                    scalar2=None,
                    op0=Alu.is_equal,
                )
            sel4 = selp.tile([P, Q, C], bf16, tag="sel")
            nc.vector.tensor_tensor(
                out=sel4[:], in0=mask4[:], in1=rgbc4[:], op=Alu.mult
            )
            # partial pairwise adds at DVE 2x before the 1x reduce.
            # exact: the one-hot product has a single nonzero, so every
            # bf16 add is 0 + x.
            s4 = sel4[:].rearrange("p q (j c) -> p q j c", j=3)
            h32 = selp.tile([P, Q, 3, 32], bf16, tag="h32")
            nc.vector.tensor_tensor(
                out=h32[:], in0=s4[:, :, :, :32], in1=s4[:, :, :, 32:], op=Alu.add
            )
            h16 = selp.tile([P, Q, 3, 16], bf16, tag="h16")
            nc.vector.tensor_tensor(
                out=h16[:], in0=h32[:, :, :, :16], in1=h32[:, :, :, 16:], op=Alu.add
            )
            nc.vector.tensor_reduce(
                rgb_all[:, t0 : t0 + Q, :],
                h16[:],
                axis=Axis.X,
                op=Alu.add,
            )

    # chunked output DMA so the store drains progressively
    OCH = 8
    och_t = T // OCH
    for o in range(OCH):
        nc.sync.dma_start(
            OUTv[:, o * och_t : (o + 1) * och_t, :],
            rgb_all[:, o * och_t : (o + 1) * och_t, :],
        )


def _reference_np(X, cluster_ids, W_pos, W_feat):
    rgbc = X[:, :POS] @ W_pos.T + X[:, POS:] @ W_feat.T
    cols = 3 * cluster_ids[:, None] + np.arange(3)[None, :]
    return np.take_along_axis(rgbc, cols, axis=1)


LAST_EXEC_NS = None


def kernel(**inputs) -> np.ndarray:
    global LAST_EXEC_NS
    from concourse.bass_utils import run_bass_kernel_spmd

    X = np.ascontiguousarray(inputs["X"], dtype=np.float32)
    cid = np.ascontiguousarray(inputs["cluster_ids"], dtype=np.int32)
    W_pos = np.ascontiguousarray(inputs["W_pos"], dtype=np.float32)
    W_feat = np.ascontiguousarray(inputs["W_feat"], dtype=np.float32)

    N = X.shape[0]
    rows = N // N_CORES
    T = rows // P
    nc = build_kernel(T)

    in_maps = []
    for c in range(N_CORES):
        sl = slice(c * rows, (c + 1) * rows)
        in_maps.append(
            {"x": X[sl], "cid": cid[sl], "w_pos": W_pos, "w_feat": W_feat}
        )
    trace = bool(int(os.environ.get("KM_TRACE", "0")))
    res = run_bass_kernel_spmd(
        nc, in_maps, core_ids=list(range(N_CORES)), trace=trace
    )
    LAST_EXEC_NS = res.exec_time_ns
    out = np.concatenate([res.results[c]["out"] for c in range(N_CORES)], axis=0)
    return out.astype(np.float32)


if __name__ == "__main__":
    # quick small-scale HW smoke test
    T = int(os.environ.get("DEV_T", "8"))
    rows = P * T * N_CORES
    rng = np.random.default_rng(0)
    X = rng.standard_normal((rows, K)).astype(np.float32)
    cid = rng.integers(0, 64, size=rows).astype(np.int32)
    W_pos = (rng.standard_normal((C, POS)) * 0.1).astype(np.float32)
    W_feat = (rng.standard_normal((C, LAT)) * 0.1).astype(np.float32)
    out = kernel(X=X, cluster_ids=cid, W_pos=W_pos, W_feat=W_feat)
    ref = _reference_np(X, cid, W_pos, W_feat)
    err = np.abs(out - ref).max() / np.abs(ref).max()
    print("max-abs relative error:", err)
